# revision 25
# baseline (speedup 1.0000x reference)
"""Trainium2 Bass kernel for nn_Attention (B=4, S=2048, HIDDEN=768, 12 heads).

Sharding: 8 cores = 4 batches x 2 head-groups (6 heads each). Projection
weights are sliced per head-group and pre-transposed on the host. Each core
computes a partial output (its head-group's contribution through Wo, with
bo/2 bias); the host sums the two partials per batch.

q/k projections run in fp8e4 DoubleRow mode (two 128-channel chunks per
pass, 2 MACs/cell/cycle). fp8e4 normals bottom out at 2^-6, so the host
scales Wq/Wk (std 0.02) by 32 before quantizing; the 1/(32*32) and the
attention 1/sqrt(64) are folded into the exp activation's free scale
multiplier (2^-13 exactly). The v projection and the attention inner
products stay bf16: v noise feeds the softmax averaging amplification
directly, and all-bf16 attention keeps PE duty high enough that the HAM
clock gate stays at full rate.

One PSUM pool is shared by all phases (tags S/A, [128, QT] tiles) so the
attention pipeline starts the moment the last projection tile drains.
Normalization runs per head-pair inside the attention window: a single
65-row PSUM evacuation per head, denominator rows DMA-reshaped to [32, 64]
(so the Newton-iteration reciprocal costs 0.4us, not 6.5us), reciprocals
DMA-broadcast via DRAM, and the qh0 output-projection tiles fill the qh1
head-pair-boundary PE stalls that would otherwise re-throttle the clock.
"""

import numpy as np
import ml_dtypes

HIDDEN = 768
NUM_HEADS = 12
D = 64
B = 4
S = 2048
P = 128

H_CORE = 6          # heads per core
OC = H_CORE * D     # 384 output channels per core for q/k
WAUG = H_CORE * (D + 1)  # 390: v columns with interleaved ones-columns
C_CHUNKS = HIDDEN // P   # 6
TC = HIDDEN // 256  # 3 DoubleRow contraction chunks (256 channels each)
QT = 1024           # q-tile (free dim) for the attention inner loop
MMF = 512           # max fp32 moving free dim for a PSUM-bank matmul
WSCALE = 32.0       # fp8 pre-scale for Wq/Wk (keeps weights out of subnormals)
EXP_SCALE = float(2.0 ** -13)  # 1/(32*32) * 1/sqrt(64)

_CACHE = {}


def _build(with_mask: bool):
    import concourse.bass as bass
    import concourse.tile as tile
    from concourse import bacc, mybir
    from contextlib import ExitStack

    f32 = mybir.dt.float32
    bf16 = mybir.dt.bfloat16
    f8 = mybir.dt.float8e4
    AF = mybir.ActivationFunctionType
    ALU = mybir.AluOpType
    DR = mybir.MatmulPerfMode.DoubleRow

    nc = bacc.Bacc(
        "TRN2",
        target_bir_lowering=False,
        debug=False,
        enable_asserts=True,
        num_devices=8,
    )

    # x8: fp8 x in DoubleRow layout [p, t, j, s] = x[256t + 128j + p, s]
    x8_d = nc.dram_tensor("x8", (P, TC, 2, S), f8, kind="ExternalInput").ap()
    wq_d = nc.dram_tensor("wq8", (P, TC, 2, OC), f8, kind="ExternalInput").ap()
    bq_d = nc.dram_tensor("bq", (OC,), f32, kind="ExternalInput").ap()
    wk_d = nc.dram_tensor("wk8", (P, TC, 2, OC), f8, kind="ExternalInput").ap()
    bk_d = nc.dram_tensor("bk", (OC,), f32, kind="ExternalInput").ap()
    # wv8: fp8 DR layout [p, t, j, col] = 32*WvT_aug[256t + 128j + p, col]
    wv_d = nc.dram_tensor("wv8", (P, TC, 2, WAUG), f8, kind="ExternalInput").ap()
    bvb_d = nc.dram_tensor("bvb", (P, WAUG), f32, kind="ExternalInput").ap()
    wo_d = nc.dram_tensor("woT", (OC, HIDDEN), bf16, kind="ExternalInput").ap()
    bo_d = nc.dram_tensor("bo", (HIDDEN,), f32, kind="ExternalInput").ap()
    if with_mask:
        em_d = nc.dram_tensor("expmask", (S, S), f32, kind="ExternalInput").ap()
    out_d = nc.dram_tensor("out", (HIDDEN, S), f32, kind="ExternalOutput").ap()

    QTILES = OC // P      # 3 q/k sbuf tiles
    STILES = S // P       # 16 s-position chunks
    NQH = S // QT         # 2 q-halves
    NF = QT // MMF        # 2 matmul free-slices per QT

    wo_r = wo_d.rearrange("(t p) o -> p t o", p=P)
    bq_r = bq_d.rearrange("(t p) -> p t", p=P)
    bk_r = bk_d.rearrange("(t p) -> p t", p=P)
    bo_r = bo_d.rearrange("(t p) -> p t", p=P)
    out_r = out_d.rearrange("(t p) s -> p t s", p=P)

    with tile.TileContext(nc) as tc, ExitStack() as ctx:
        consts = ctx.enter_context(tc.tile_pool(name="consts", bufs=1))
        persist = ctx.enter_context(tc.tile_pool(name="persist", bufs=1))
        phB = ctx.enter_context(tc.tile_pool(name="phB", bufs=4))
        psm = ctx.enter_context(tc.tile_pool(name="psm", bufs=2, space="PSUM"))
        outp = ctx.enter_context(tc.tile_pool(name="outp", bufs=2))
        dscr = ctx.enter_context(tc.tile_pool(name="dscr", bufs=3, space="DRAM"))

        bvb_t = consts.tile([P, WAUG], f32)
        nc.sync.dma_start(bvb_t[:], bvb_d)
        bq_t = consts.tile([P, QTILES], f32)
        nc.sync.dma_start(bq_t[:], bq_r)
        bk_t = consts.tile([P, QTILES], f32)
        nc.sync.dma_start(bk_t[:], bk_r)
        bo_t = consts.tile([P, C_CHUNKS], f32)
        nc.sync.dma_start(bo_t[:], bo_r)

        q_t = persist.tile([P, QTILES, S], bf16)
        k_t = persist.tile([P, QTILES, S], bf16)
        v_t = persist.tile([P, STILES, WAUG], bf16)
        attn_t = persist.tile([P, QTILES, S], bf16)
        wo_t = persist.tile([P, QTILES, HIDDEN], bf16)
        x8_t = persist.tile([P, TC, 2, S], f8)
        wq_t = persist.tile([P, TC, 2, OC], f8)
        wk_t = persist.tile([P, TC, 2, OC], f8)
        wv_t = persist.tile([P, TC, 2, WAUG], f8)

        # ---------------- phase A: projections ----------------
        # DMA priority order: what the attention head needs lands first
        # (x8 + q/k weights + wv + the first xb s-half); the xb tail and wo
        # follow. All 16 DMA queues round-robin in emission order.
        for t in range(TC):
            nc.sync.dma_start(x8_t[:, t, :, :], x8_d[:, t, :, :])
        nc.sync.dma_start(wq_t[:], wq_d)
        nc.sync.dma_start(wk_t[:], wk_d)
        nc.sync.dma_start(wv_t[:], wv_d)
        # Startup warmup: the PE starts HAM-throttled at 1.2 GHz and only
        # un-throttles after ~3.4us of sustained activity. Burn that time on
        # fp32 matmuls (4 passes each) while the x/w DMAs land.
        wu0 = psm.tile([P, QT], f32, tag="S", name="warmup0")
        for i in range(4):
            nc.tensor.matmul(
                wu0[:, 0:390], bvb_t[:, 0:P], bvb_t[:],
                start=True, stop=True,
            )
        nc.sync.dma_start(wo_t[:], wo_r)

        def qk_quantum(which, ot, half, tg="S"):
            # One q/k projection tile (fp8 DoubleRow): [128, 1024] PSUM
            # accumulating 3 DoubleRow passes over the 768-channel
            # contraction, then a bias-add evacuation to bf16.
            dst, w_sb, b_sb = ((q_t, wq_t, bq_t) if which == "q"
                               else (k_t, wk_t, bk_t))
            ps = psm.tile([P, QT], f32, tag=tg,
                          name=f"pw_{which}{ot}_{half}")
            for t in range(TC):
                for nf in range(NF):
                    nc.tensor.matmul(
                        ps[:, nf * MMF:(nf + 1) * MMF],
                        w_sb[:, t, :, ot * P:(ot + 1) * P],
                        x8_t[:, t, :,
                             half * QT + nf * MMF:
                             half * QT + (nf + 1) * MMF],
                        start=(t == 0),
                        stop=(t == TC - 1),
                        perf_mode=DR,
                    )
            nc.vector.tensor_scalar_add(
                dst[:, ot, half * QT:(half + 1) * QT],
                ps[:],
                b_sb[:, ot:ot + 1],
            )

        def v_quantum(sp):
            # vT projection (fp8 DoubleRow, 32x-prescaled weights; the 32
            # cancels between the attnV numerator and the ones-column
            # denominator) for s-chunks 2sp, 2sp+1: two 390-wide groups per
            # [128, 1024] PSUM tile (at cols 0 and 512).
            ps = psm.tile([P, QT], f32, tag="S", name=f"pv{sp}")
            for g in range(2):
                st = 2 * sp + g
                for t in range(TC):
                    nc.tensor.matmul(
                        ps[:, g * MMF:g * MMF + WAUG],
                        x8_t[:, t, :, st * P:(st + 1) * P],
                        wv_t[:, t, :, :],
                        start=(t == 0),
                        stop=(t == TC - 1),
                        perf_mode=DR,
                    )
            for g in range(2):
                nc.vector.tensor_tensor(
                    v_t[:, 2 * sp + g, :],
                    ps[:, g * MMF:g * MMF + WAUG],
                    bvb_t[:],
                    ALU.add,
                )

        # head-of-kernel projections: what attention head-pair 0 needs
        # early (q/k tile 0, v s-chunks 0-7); the rest interleaves into the
        # attention window as PE-slack fillers, each just-in-time for its
        # first consumer.
        qk_quantum("q", 0, 0, "S")
        qk_quantum("k", 0, 0, "A")
        qk_quantum("k", 0, 1, "S")
        for sp in range(8):
            v_quantum(sp)

        # (qh, hp, c) -> projection quantum to emit after that chunk;
        # (qh, hp) -> quantum to emit at that head-pair boundary (where the
        # exp stream stalls anyway, so the quantum's handover rides free).
        fillers = {
            (0, 0, 12): (qk_quantum, ("k", 1, 0)),
            (0, 0, 14): (qk_quantum, ("q", 1, 0)),
            (0, 1, 2): (qk_quantum, ("k", 2, 0)),
            (0, 1, 6): (qk_quantum, ("q", 2, 0)),
            (0, 2, 4): (qk_quantum, ("q", 2, 1)),
        }
        bfillers = {
            (0, 0): [(qk_quantum, ("k", 1, 1))],
            (0, 1): [(qk_quantum, ("k", 2, 1)),
                     (qk_quantum, ("q", 0, 1))],
            (0, 2): [(qk_quantum, ("q", 1, 1))],
        }
        late_fillers = {}

        # ---------------- phase B: attention ----------------
        def emit_outproj(qh, ot, dve_bias=False):
            ps = psm.tile([P, QT], f32, tag="S", name=f"po{qh}_{ot}")
            for ct in range(QTILES):
                for nf in range(NF):
                    nc.tensor.matmul(
                        ps[:, nf * MMF:(nf + 1) * MMF],
                        wo_t[:, ct, ot * P:(ot + 1) * P],
                        attn_t[:, ct, qh * QT + nf * MMF:
                               qh * QT + (nf + 1) * MMF],
                        start=(ct == 0),
                        stop=(ct == QTILES - 1),
                    )
            o_sb = outp.tile([P, QT], f32, tag="O", name=f"ob{qh}_{ot}")
            if dve_bias:
                nc.vector.tensor_scalar_add(o_sb[:], ps[:], bo_t[:, ot:ot + 1])
            else:
                nc.scalar.add(o_sb[:], ps[:], bo_t[:, ot:ot + 1])
            nc.sync.dma_start(out_r[:, ot, qh * QT:(qh + 1) * QT], o_sb[:])

        late_fillers[(1, 2, 4)] = (emit_outproj, (0, 4))
        late_fillers[(1, 2, 8)] = (emit_outproj, (0, 5))

        def warm(n, name):
            wub = psm.tile([P, QT], f32, tag="S", name=name)
            for i in range(n):
                nc.tensor.matmul(
                    wub[:, 0:MMF], q_t[:, 0, 0:P], q_t[:, 0, 0:MMF],
                    start=True, stop=True,
                )

        parts = []
        for qh in range(NQH):
            for hp in range(H_CORE // 2):
                heads = (2 * hp, 2 * hp + 1)
                accs = [
                    psm.tile([P, QT], f32, tag="A", name=f"acc{qh}_{hp}_{i}")
                    for i in range(2)
                ]
                for c in range(STILES):
                    etiles = []
                    for hi, h in enumerate(heads):
                        pb = 64 * (h % 2)
                        sc = psm.tile([P, QT], f32, tag="S",
                                      name=f"sc{qh}_{hp}_{c}_{hi}")
                        for nf in range(NF):
                            nc.tensor.matmul(
                                sc[:, nf * MMF:(nf + 1) * MMF],
                                k_t[pb:pb + D, h // 2, c * P:(c + 1) * P],
                                q_t[pb:pb + D, h // 2,
                                    qh * QT + nf * MMF:
                                    qh * QT + (nf + 1) * MMF],
                                start=True,
                                stop=True,
                            )
                        e = phB.tile([P, QT], bf16, tag="E")
                        nc.scalar.activation(e[:], sc[:], AF.Exp,
                                             scale=EXP_SCALE)
                        if with_mask:
                            em = phB.tile([P, QT], f32, tag="M")
                            nc.sync.dma_start(
                                em[:],
                                em_d[c * P:(c + 1) * P,
                                     qh * QT:(qh + 1) * QT],
                            )
                            nc.vector.tensor_tensor(
                                e[:], e[:], em[:], ALU.mult
                            )
                        etiles.append(e)
                    for hi, h in enumerate(heads):
                        for nf in range(NF):
                            nc.tensor.matmul(
                                accs[hi][0:D + 1, nf * MMF:(nf + 1) * MMF],
                                v_t[:, c, 65 * h:65 * h + 65],
                                etiles[hi][:, nf * MMF:(nf + 1) * MMF],
                                start=(c == 0),
                                stop=(c == STILES - 1),
                            )
                    if (qh, hp, c) in fillers:
                        fn, args = fillers[(qh, hp, c)]
                        fn(*args)
                    if (qh, hp, c) in late_fillers:
                        fn, args = late_fillers[(qh, hp, c)]
                        fn(*args)
                # per-head-pair normalization, spread into the next hp's
                # attention window. Single 65-row evacuation frees the acc
                # PSUM fast; denominator rows (row 64) are DMA-reshaped to
                # 16 partitions x 64 so the reciprocal runs 16x faster.
                dens = phB.tile([32, D, 1], f32, tag="dn", bufs=3,
                                name=f"dn{qh}_{hp}")
                u65s = []
                for hi, h in enumerate(heads):
                    u65 = phB.tile([D + 1, QT], f32, tag="U", bufs=6,
                                   name=f"u{qh}_{h}")
                    if hi == 0:
                        nc.vector.tensor_copy(u65[:], accs[hi][0:D + 1, :])
                    else:
                        nc.scalar.copy(u65[:], accs[hi][0:D + 1, :])
                    u65s.append(u65)
                for hi, h in enumerate(heads):
                    nc.sync.dma_start(dens[16 * hi:16 * hi + 16, :, 0],
                                      u65s[hi][D:D + 1, :])
                # hp-boundary PE fillers: in qh1 the filler is qh0's output
                # projection (ScalarE bias-add — the exp stream is stalled
                # here anyway, and DVE must stay clear of the S-slot path);
                # in qh0 a leftover projection quantum.
                if qh == 1 and hp < 2:
                    emit_outproj(0, 2 * hp)
                    emit_outproj(0, 2 * hp + 1)
                for fn, args in bfillers.get((qh, hp), []):
                    fn(*args)
                if (qh, hp) == (1, 2):
                    # tail overlap: the first two contraction chunks of the
                    # qh1 output projection run during the norm chain's DMA
                    # latency; only ct2 remains afterwards. ot0/ot1 stay open
                    # in the (free) acc-ring PSUM so ct2 accumulates in place
                    # (one ScalarE evac, no DVE combine); ot2-5 evacuate via
                    # ScalarE to SBUF partials and recombine on DVE.
                    for ot in range(C_CHUNKS):
                        pp = psm.tile([P, QT], f32,
                                      tag=("A" if ot < 2 else "S"),
                                      name=f"pp{ot}")
                        for ct in range(2):
                            for nf in range(NF):
                                nc.tensor.matmul(
                                    pp[:, nf * MMF:(nf + 1) * MMF],
                                    wo_t[:, ct, ot * P:(ot + 1) * P],
                                    attn_t[:, ct, QT + nf * MMF:
                                           QT + (nf + 1) * MMF],
                                    start=(ct == 0),
                                    stop=(ct == 1 and ot >= 2),
                                )
                        if ot < 2:
                            parts.append(pp)
                        else:
                            part = outp.tile([P, QT], f32, tag="PP", bufs=4,
                                             name=f"part{ot}")
                            nc.scalar.add(part[:], pp[:], bo_t[:, ot:ot + 1])
                            parts.append(part)
                rec = phB.tile([32, D, 1], f32, tag="rc", bufs=3,
                               name=f"rc{qh}_{hp}")
                nc.vector.reciprocal(rec[:], dens[:])
                scr2 = dscr.tile([2, QT], f32, tag="sc2", name=f"s2_{qh}{hp}")
                for hi, h in enumerate(heads):
                    nc.sync.dma_start(scr2[hi:hi + 1, :],
                                      rec[16 * hi:16 * hi + 16, :, 0])
                for hi, h in enumerate(heads):
                    bc = phB.tile([D, QT], f32, tag="B")
                    nc.sync.dma_start(
                        bc[:], scr2[hi:hi + 1, :].to_broadcast((D, QT))
                    )
                    pb = 64 * (h % 2)
                    nc.vector.tensor_tensor(
                        attn_t[pb:pb + D, h // 2, qh * QT:(qh + 1) * QT],
                        u65s[hi][0:D, :],
                        bc[:],
                        ALU.mult,
                    )
                # HAM insurance: keep the PE from idling through the rest of
                # the boundary stall.
                if (qh, hp) != (1, 2):
                    warm(5 if qh == 0 else 6, f"wub{qh}_{hp}")
                else:
                    warm(4, f"wub{qh}_{hp}")

        # ---------------- output projection (tail) ----------------
        for ot in range(C_CHUNKS):
            if ot < 2:
                pp = parts[ot]
                for nf in range(NF):
                    nc.tensor.matmul(
                        pp[:, nf * MMF:(nf + 1) * MMF],
                        wo_t[:, 2, ot * P:(ot + 1) * P],
                        attn_t[:, 2, QT + nf * MMF:QT + (nf + 1) * MMF],
                        start=False,
                        stop=True,
                    )
                o_sb = outp.tile([P, QT], f32, tag="O", name=f"of{ot}")
                nc.scalar.add(o_sb[:], pp[:], bo_t[:, ot:ot + 1])
            else:
                ps = psm.tile([P, QT], f32, tag="S", name=f"pf{ot}")
                for nf in range(NF):
                    nc.tensor.matmul(
                        ps[:, nf * MMF:(nf + 1) * MMF],
                        wo_t[:, 2, ot * P:(ot + 1) * P],
                        attn_t[:, 2, QT + nf * MMF:QT + (nf + 1) * MMF],
                        start=True,
                        stop=True,
                    )
                o_sb = outp.tile([P, QT], f32, tag="O", name=f"of{ot}")
                nc.vector.tensor_tensor(o_sb[:], ps[:], parts[ot][:],
                                        ALU.add)
            nc.sync.dma_start(out_r[:, ot, QT:S], o_sb[:])

    nc.compile()
    return nc


def _get_program(with_mask: bool):
    key = ("prog", with_mask)
    if key not in _CACHE:
        _CACHE[key] = _build(with_mask)
    return _CACHE[key]


def _prep_inputs(hidden_state, mask, Wq, bq, Wk, bk, Wv, bv, Wo, bo):
    """Build the 8 per-core input dicts (host-side shard + weight prep)."""
    f = np.float32
    f8 = ml_dtypes.float8_e4m3
    bf = ml_dtypes.bfloat16
    with_mask = bool(np.any(mask))
    ws = np.float32(WSCALE)

    def dr_layout(wT):
        # [HIDDEN, O] -> [P, TC, 2, O] with channel (256t + 128j + p)
        return np.ascontiguousarray(
            wT.reshape(TC, 2, P, -1).transpose(2, 0, 1, 3))

    in_maps = []
    for b in range(B):
        x_b = np.asarray(hidden_state[b, :, 0, :], dtype=f)
        x8 = dr_layout(x_b).astype(f8)
        if with_mask:
            em_b = np.exp(mask[b, :, 0, :].astype(f))
        for g in range(2):
            rows = slice(OC * g, OC * (g + 1))
            wq8 = dr_layout(np.asarray(Wq[rows, :], dtype=f).T * ws).astype(f8)
            bqs = np.ascontiguousarray(np.asarray(bq[rows], dtype=f) * ws)
            wk8 = dr_layout(np.asarray(Wk[rows, :], dtype=f).T * ws).astype(f8)
            bks = np.ascontiguousarray(np.asarray(bk[rows], dtype=f) * ws)
            # augmented v weights: col 65h+j = Wv row, col 65h+64 = 0 (bias 1)
            wvT = np.zeros((HIDDEN, WAUG), dtype=f)
            bvb = np.zeros((WAUG,), dtype=f)
            for h in range(H_CORE):
                wvT[:, 65 * h:65 * h + 64] = np.asarray(
                    Wv[OC * g + D * h:OC * g + D * h + D, :], dtype=f).T
                bvb[65 * h:65 * h + 64] = np.asarray(
                    bv[OC * g + D * h:OC * g + D * h + D], dtype=f) * ws
                bvb[65 * h + 64] = ws
            woT = (np.asarray(Wo[:, rows], dtype=f).T).astype(bf)
            m = {
                "x8": x8,
                "wq8": wq8,
                "bq": bqs,
                "wk8": wk8,
                "bk": bks,
                "wv8": dr_layout(wvT * ws).astype(f8),
                "bvb": np.broadcast_to(bvb, (P, WAUG)).copy(),
                "woT": woT,
                "bo": (np.asarray(bo, dtype=f) * np.float32(0.5)),
            }
            if with_mask:
                m["expmask"] = em_b
            in_maps.append(m)
    return in_maps, with_mask


def run(inputs: dict, trace: bool = False):
    """Run on 8 NeuronCores; returns (full_output, exec_time_ns_or_None)."""
    from concourse import bass_utils

    in_maps, with_mask = _prep_inputs(**inputs)
    nc = _get_program(with_mask)
    res = bass_utils.run_bass_kernel_spmd(
        nc, in_maps, core_ids=list(range(8)), trace=trace
    )
    out = np.empty((B, HIDDEN, 1, S), dtype=np.float32)
    for b in range(B):
        out[b, :, 0, :] = res.results[2 * b]["out"] + res.results[2 * b + 1]["out"]
    return out, res.exec_time_ns


def kernel(**inputs) -> np.ndarray:
    out, _ = run(inputs, trace=False)
    return out
